# revision 1
# baseline (speedup 1.0000x reference)
"""Trainium2 Bass kernel for fused embedding-lookup -> mean-pool -> dot(weights).

Reference computation (B=16384, L=200, D=100, V=100000):
    out[b] = mean_l(embed_table[word_idxs[b, l], :]) @ weights            # [B, 1]

Key algebraic transform: the dot with `weights` is linear, so
    out[b] = sum_l s[word_idxs[b, l]],   with  s = embed_table @ (weights / L)
Instead of gathering B*L rows of 400B (1.31 GB), we precompute the V-element
vector `s` on-device (the 40MB table is read exactly once across the 8 cores)
and gather B*L scalars.

The scalar gather uses the TIE-ucode `dma_gather` (int16 row indices, 256B
elements, 4 SWDGE queues). To avoid a 64-wide on-chip select per token, we
materialize a phase-shifted fat-row table in DRAM:
    S16[j, k] = s_pad[4*j + k],  j in [0, 25000), k in [0, 64)
(dense 256B rows; s_pad = s with a 32-element zero lead pad). For a token with
index v, row j = v >> 2 contains s[v] at lane 30 + (v & 3) — a fixed 4-lane
window, so the select is a 4-wide mask+reduce (~40us DVE total).

Sharding (8 cores): batch-parallel gather (2048 rows/core); vocab-parallel s
precompute (12544 padded rows/core) + AllGather.

Host does layout only: shard/reshape inputs, compute j = idx>>2 / r = idx&3,
wrap indices in the dma_gather [16, S] layout, and concat per-core outputs.
"""

import os
import sys

import numpy as np

for _p in ("/opt/trn_rl_repo",):
    if os.path.isdir(_p) and _p not in sys.path:
        sys.path.insert(0, _p)

from concourse import bacc, bass, mybir, tile  # noqa: E402
from concourse.bass_utils import run_bass_kernel_spmd  # noqa: E402

F32 = mybir.dt.float32
I32 = mybir.dt.int32
I16 = mybir.dt.int16
P = 128
NCORES = 8


def dma_gather_raw(
    gp, out_ap, in_ap, idxs_ap, num_idxs, num_idxs_reg, elem_size, elem_step,
    queue_num=0,
):
    """nc.gpsimd.dma_gather minus the 256B *element* restriction.

    Only the source row PITCH must be a 256B multiple (stride_bytes_256 is an
    8-bit field in 256B units); the per-index element payload can be smaller.
    Emits the same InstDMAGatherAnt the stock wrapper does.
    """
    dt_sz = mybir.dt.size(in_ap.dtype)
    stride_256 = (elem_step * dt_sz) // 256
    assert elem_step * dt_sz == stride_256 * 256 and 0 < stride_256 < 256
    assert in_ap.ap[0][0] == elem_step and in_ap.ap[-1][1] == elem_size
    _in_ap = gp.lower_ap_dma(in_ap, for_custom_bir_dma=True)
    _idxs_ap = gp.lower_ap(idxs_ap)
    _out_ap = gp.lower_ap(out_ap)
    return gp.add_instruction(
        mybir.InstDMAGatherAnt(
            name=gp.bass.get_next_instruction_name(),
            ins=[*_in_ap, _idxs_ap, gp.lower_val_access(gp.to_reg(num_idxs_reg))],
            outs=[_out_ap],
            transpose=False,
            num_idxs=num_idxs,
            elem_size=elem_size,
            stride_bytes_256=stride_256,
            gen_mode=0,
            single_packet=False,
            queue_num=queue_num,
            sbuf_tokens_per_rank=0,
            sbuf_free_dim_per_rank=0,
            sbuf_free_dim_pad_per_rank=0,
            sbuf_byte_offset=0,
        )
    )


def build_program(
    G=16, L=200, D=100, RPP=98, CPI=100, NQ=4, ncores=NCORES, use_collective=True,
    repeat=1, ELEM=4, GAT_BUFS=2,
):
    """Build the SPMD program (identical on all cores).

    G:   row-groups per core (batch rows per core = G*128)
    L:   tokens per row
    D:   embedding dim
    RPP: padded vocab rows per SBUF partition (vocab rows per core = 128*RPP)
    CPI: token slots (out columns) per dma_gather instruction; L % CPI == 0
    NQ:  SWDGE queues to rotate over (1..4)
    """
    assert L % CPI == 0
    SLOTS = G * L  # token slots per partition
    NT = SLOTS // CPI  # dma_gather instructions
    H = L // CPI  # instructions per row-group
    NI = P * CPI  # indices per instruction
    VPC = P * RPP
    V_PAD = VPC * ncores
    NROWS = V_PAD // 4  # row j holds s[4j .. 4j+4)
    nc = bacc.Bacc(
        "TRN2",
        target_bir_lowering=False,
        debug=False,
        num_devices=ncores,
        num_swdge_queues=NQ,
    )
    idxw_t = nc.dram_tensor("idxw", [P, SLOTS * 8], I16, kind="ExternalInput")
    io4_t = nc.dram_tensor("io4", [P, 4], F32, kind="ExternalInput")
    r2_t = nc.dram_tensor("r2", [P, SLOTS], F32, kind="ExternalInput")
    tab_t = nc.dram_tensor("tab", [P, RPP * D], F32, kind="ExternalInput")
    w_t = nc.dram_tensor("w", [P, D], F32, kind="ExternalInput")
    out_t = nc.dram_tensor("out", [P, G], F32, kind="ExternalOutput")

    with tile.TileContext(nc) as tc:
        with tc.tile_pool(name="dr", bufs=1, space="DRAM") as dr:
            with tc.tile_pool(name="pre", bufs=1) as pre:
                # ---- stage 1: s_part = (table slice) @ (w/L) ----
                tab_sb = pre.tile([P, RPP * D], F32)
                nc.sync.dma_start(tab_sb[:], tab_t[:])
                w_sb = pre.tile([P, D], F32)
                nc.sync.dma_start(w_sb[:], w_t[:])
                prod_sb = pre.tile([P, RPP * D], F32)
                nc.vector.tensor_tensor(
                    out=prod_sb[:].rearrange("p (r d) -> p r d", d=D),
                    in0=tab_sb[:].rearrange("p (r d) -> p r d", d=D),
                    in1=w_sb[:].unsqueeze(1).to_broadcast([P, RPP, D]),
                    op=mybir.AluOpType.mult,
                )
                s_sb = pre.tile([P, RPP], F32)
                nc.vector.tensor_reduce(
                    out=s_sb[:].unsqueeze(2),
                    in_=prod_sb[:].rearrange("p (r d) -> p r d", d=D),
                    axis=mybir.AxisListType.X,
                    op=mybir.AluOpType.add,
                )

                # ---- stage 2: AllGather s ----
                s_part = dr.tile([P, RPP], F32)
                nc.sync.dma_start(s_part[:], s_sb[:])
                if use_collective:
                    s_full = dr.tile([ncores * RPP, P], F32, addr_space="Shared")
                    nc.gpsimd.collective_compute(
                        "AllGather",
                        mybir.AluOpType.bypass,
                        replica_groups=[list(range(ncores))],
                        ins=[s_part.opt()],
                        outs=[s_full.opt()],
                    )
                else:
                    # crash-isolation mode: fill s_full with the local part
                    # replicated (wrong data, same program shape)
                    s_full = dr.tile([ncores * RPP, P], F32)
                    for c in range(ncores):
                        nc.sync.dma_start(
                            s_full[c * RPP : (c + 1) * RPP, :],
                            s_part[:].rearrange("p r -> (p r)").rearrange(
                                "(r q) -> r q", q=P
                            ),
                        )

                # ---- stage 3: spread s into 256B-pitch rows ----
                # S16[j, 0:4] = s[4j .. 4j+4); rows pitched 64 f32 = 256B so
                # the gather row stride is ISA-encodable; lanes 4..63 are
                # never written nor read. Token v -> row v>>2, lane v&3.
                S16 = dr.tile([NROWS, 64], F32)
                s_flat = s_full[:].rearrange("a b -> (a b)")
                # chunk: large descriptor counts in one SWDGE dma_start
                # overflow the SDMA packet machinery (HW crash above ~1k
                # descriptors per instruction).
                row = 0
                while row < NROWS:
                    n = min(1000, NROWS - row)
                    src_view = bass.AP(s_flat.tensor, 4 * row, [[4, n], [1, 4]])
                    nc.sync.dma_start(S16[row : row + n, 0:4], src_view)
                    row += n

            with (
                tc.tile_pool(name="keep", bufs=1) as keep,
                tc.tile_pool(name="gat", bufs=GAT_BUFS) as gat,
            ):
                # ---- stage 4: gather + select + reduce ----
                iota4 = keep.tile([P, 4], F32)
                nc.sync.dma_start(iota4[:], io4_t[:])
                r2_sb = keep.tile([P, SLOTS], F32)
                nc.sync.dma_start(r2_sb[:], r2_t[:])
                half_sb = keep.tile([P, NT], F32)
                out_sb = keep.tile([P, G], F32)
                iota_view = iota4[:].unsqueeze(1).to_broadcast([P, CPI, 4])
                for t in range(NT * repeat):
                    t = t % NT
                    idxw_sb = gat.tile([P, NI // 16], I16, tag="idxw", name=f"idxw{t}")
                    nc.sync.dma_start(
                        idxw_sb[:], idxw_t[:, t * (NI // 16) : (t + 1) * (NI // 16)]
                    )
                    gth = gat.tile([P, CPI, ELEM], F32, tag="gth", name=f"gth{t}")
                    dma_gather_raw(
                        nc.gpsimd,
                        gth[:],
                        S16[:, 0:ELEM],
                        idxw_sb[:],
                        NI,
                        NI,
                        elem_size=ELEM,
                        elem_step=64,
                        queue_num=t % NQ,
                    )
                    mask = gat.tile([P, CPI, 4], F32, tag="mask", name=f"mask{t}")
                    nc.vector.tensor_tensor(
                        out=mask[:],
                        in0=r2_sb[:, t * CPI : (t + 1) * CPI]
                        .unsqueeze(2)
                        .to_broadcast([P, CPI, 4]),
                        in1=iota_view,
                        op=mybir.AluOpType.is_equal,
                    )
                    msel = gat.tile([P, CPI, 4], F32, tag="msel", name=f"msel{t}")
                    nc.vector.tensor_tensor(
                        out=msel[:],
                        in0=mask[:],
                        in1=gth[:, :, 0:4],
                        op=mybir.AluOpType.mult,
                    )
                    nc.vector.tensor_reduce(
                        out=half_sb[:, t : t + 1],
                        in_=msel[:].rearrange("p a b -> p (a b)"),
                        axis=mybir.AxisListType.X,
                        op=mybir.AluOpType.add,
                    )
                nc.vector.tensor_reduce(
                    out=out_sb[:].unsqueeze(2),
                    in_=half_sb[:].rearrange("p (g h) -> p g h", h=H),
                    axis=mybir.AxisListType.X,
                    op=mybir.AluOpType.add,
                )
                nc.sync.dma_start(out_t[:], out_sb[:])
    nc.compile()
    return nc


def make_in_maps(word_idxs, embed_table, weights, G, L, D, RPP, CPI, ncores=NCORES):
    """Shard + lay out the full inputs for the per-core program."""
    BPC = G * P
    SLOTS = G * L
    NT = SLOTS // CPI
    VPC = P * RPP
    idx = np.asarray(word_idxs).astype(np.int32)
    tab = np.asarray(embed_table, dtype=np.float32)
    w = np.asarray(weights, dtype=np.float32).reshape(-1)
    V = tab.shape[0]
    tab_pad = np.zeros((VPC * ncores, D), dtype=np.float32)
    tab_pad[:V] = tab
    w_c = np.ascontiguousarray(
        np.broadcast_to((w / np.float32(L))[None, :], (P, D))
    ).astype(np.float32)
    in_maps = []
    for c in range(ncores):
        # token slot layout: [partition p, slot j=g*L+l] holds idx of batch
        # row (c*BPC + g*128 + p), token l
        slots = (
            idx[c * BPC : (c + 1) * BPC]
            .reshape(G, P, L)
            .transpose(1, 0, 2)
            .reshape(P, SLOTS)
        )
        jmat = (slots >> 2).astype(np.int16)  # [P, SLOTS]
        r2 = (slots & 3).astype(np.float32)
        # per-instruction index lists in i = c_local*128 + p order, wrapped
        # into the dma_gather [16, NI//16] layout, replicated to 128 parts
        u = jmat.reshape(P, NT, CPI).transpose(1, 2, 0)  # [NT, CPI, P]
        wrp = u.reshape(NT, CPI * P // 16, 16).transpose(2, 0, 1).reshape(16, -1)
        idxw = np.ascontiguousarray(np.tile(wrp, (8, 1)))  # [128, SLOTS*8]
        tab_c = np.ascontiguousarray(
            tab_pad[c * VPC : (c + 1) * VPC].reshape(P, RPP * D)
        )
        in_maps.append(
            {
                "idxw": idxw,
                "r2": np.ascontiguousarray(r2),
                "tab": tab_c,
                "w": w_c,
                "io4": np.ascontiguousarray(
                    np.broadcast_to(np.arange(4, dtype=np.float32), (P, 4))
                ),
            }
        )
    return in_maps


def unshard_out(results, G, ncores=NCORES):
    """results: list of per-core {'out': [128, G]} -> full [B, 1] f32."""
    parts = []
    for c in range(ncores):
        o = np.asarray(results[c]["out"])  # [P, G]; out[p, g] = row g*128+p
        parts.append(o.T.reshape(-1))
    return np.concatenate(parts).reshape(-1, 1).astype(np.float32)


_CACHED_NC = None

FULL = dict(G=16, L=200, D=100, RPP=98, CPI=100)


def _get_nc():
    global _CACHED_NC
    if _CACHED_NC is None:
        _CACHED_NC = build_program(**FULL)
    return _CACHED_NC


def run(word_idxs, embed_table, weights, trace=False, **spmd_kwargs):
    """Build (cached), run on the 8 cores, return (full_out, BassKernelResults)."""
    nc = _get_nc()
    in_maps = make_in_maps(
        word_idxs,
        embed_table,
        weights,
        FULL["G"],
        FULL["L"],
        FULL["D"],
        FULL["RPP"],
        FULL["CPI"],
    )
    res = run_bass_kernel_spmd(
        nc, in_maps, core_ids=list(range(NCORES)), trace=trace, **spmd_kwargs
    )
    out = unshard_out(res.results, FULL["G"])
    return out, res


def kernel(word_idxs, embed_table, weights):
    out, _ = run(word_idxs, embed_table, weights, trace=False)
    return out



# revision 2
# speedup vs baseline: 1.0018x; 1.0018x over previous
"""Trainium2 Bass kernel: embedding-lookup -> mean-pool -> dot(weights).

out[b] = sum_l s[idx[b,l]],  s = embed_table @ (weights/L)   (V=100000, D=100)

Gather strategy (per core, 2048 batch rows, 409,600 tokens):
  - s striped 16-way across partitions: tab16[p, e] = s[16e + (p&15)],
    e < 6272 (25KB/partition, identical stripes in each 16-partition group).
  - 8x ap_gather (GPSIMD, all 8 Q7 cores in parallel): each 16-partition
    group g gathers its own token list L_g (its 256 batch rows x 200 tokens)
    by m = v>>4; output [128, 6400] holds, at partition p, s[16*m_i + (p&15)].
  - DVE: multiply by a host-shipped bf16 lane mask (j_i == p&15), reduce
    each row's 200-token run -> rs[128, 256] partial sums per partition.
  - PE: W8^T @ rs with W8[p, m] = (p>>4 == m) sums the 16 partitions of each
    group -> psum [8, 256] = all 2048 row outputs.

Vocab-parallel s precompute (12544 rows/core, strided row assignment so the
local s chunk is already stripe-ordered) + AllGather, as in the classic
data-parallel embedding recipe. Host does layout only: row re-ordering of the
table, index splitting (v>>4, v&15), wrap layouts, concat of outputs.
"""

import os
import sys

import numpy as np

for _p in ("/opt/trn_rl_repo",):
    if os.path.isdir(_p) and _p not in sys.path:
        sys.path.insert(0, _p)

from concourse import bacc, bass, mybir, tile  # noqa: E402
from concourse.bass_utils import run_bass_kernel_spmd  # noqa: E402

F32 = mybir.dt.float32
BF16 = mybir.dt.bfloat16
I16 = mybir.dt.int16
P = 128
NCORES = 8

B, L, D, V = 16384, 200, 100, 100000
RPP = 98  # vocab rows per partition (per core): 128*98*8 = 100352 >= V
VPC = P * RPP  # 12544 vocab rows per core
NE = VPC * NCORES // 16  # 6272 stripe entries per partition
ROWS_PER_CORE = B // NCORES  # 2048
ROWS_PER_GROUP = ROWS_PER_CORE // 8  # 256
TOK_PER_GROUP = ROWS_PER_GROUP * L  # 51200
NI = 6400  # idxs per ap_gather per group (32 rows' runs)
NT = TOK_PER_GROUP // NI  # 8 gather instructions


def build_program(mask_dtype=BF16, gat_bufs=2):
    nc = bacc.Bacc(
        "TRN2", target_bir_lowering=False, debug=False, num_devices=NCORES
    )
    tab_t = nc.dram_tensor("tab", [P, RPP * D], F32, kind="ExternalInput")
    w_t = nc.dram_tensor("w", [P, D], F32, kind="ExternalInput")
    idx_t = nc.dram_tensor("idx", [P, TOK_PER_GROUP // 16], I16, kind="ExternalInput")
    msk_t = nc.dram_tensor("msk", [P, TOK_PER_GROUP], mask_dtype, kind="ExternalInput")
    w8_t = nc.dram_tensor("w8", [P, 8], F32, kind="ExternalInput")
    out_t = nc.dram_tensor("out", [8, ROWS_PER_GROUP], F32, kind="ExternalOutput")

    with tile.TileContext(nc) as tc:
        with tc.tile_pool(name="dr", bufs=1, space="DRAM") as dr:
            with tc.tile_pool(name="keep", bufs=1) as keep:
                with tc.tile_pool(name="pre", bufs=1) as pre:
                    # ---- stage 1: local striped s chunk = (table slice) @ (w/L) ----
                    tab_sb = pre.tile([P, RPP * D], F32)
                    w_sb = pre.tile([P, D], F32)
                    nc.sync.dma_start(w_sb[:], w_t[:])
                    s_sb = pre.tile([P, RPP], F32)
                    HALF = RPP // 2
                    for h in range(2):
                        r0, r1 = h * HALF, (h + 1) * HALF
                        nc.sync.dma_start(
                            tab_sb[:, r0 * D : r1 * D], tab_t[:, r0 * D : r1 * D]
                        )
                        prod_sb = pre.tile([P, HALF * D], F32, tag="prod", name=f"pr{h}")
                        nc.vector.tensor_tensor(
                            out=prod_sb[:].rearrange("p (r d) -> p r d", d=D),
                            in0=tab_sb[:, r0 * D : r1 * D].rearrange(
                                "p (r d) -> p r d", d=D
                            ),
                            in1=w_sb[:].unsqueeze(1).to_broadcast([P, HALF, D]),
                            op=mybir.AluOpType.mult,
                        )
                        nc.vector.tensor_reduce(
                            out=s_sb[:, r0:r1].unsqueeze(2),
                            in_=prod_sb[:].rearrange("p (r d) -> p r d", d=D),
                            axis=mybir.AxisListType.X,
                            op=mybir.AluOpType.add,
                        )

                    # ---- stage 2: AllGather striped s (Pool-triggered; sits
                    # behind the warm gather in the Pool queue) ----
                    s_part = dr.tile([P, RPP], F32)
                    nc.sync.dma_start(s_part[:], s_sb[:])
                    s_full = dr.tile([NCORES * RPP, P], F32, addr_space="Shared")
                    nc.gpsimd.collective_compute(
                        "AllGather",
                        mybir.AluOpType.bypass,
                        replica_groups=[list(range(NCORES))],
                        ins=[s_part.opt()],
                        outs=[s_full.opt()],
                    )

                    # ---- stage 3: load striped table into SBUF ----
                    # flat s_full element index = c*12544 + (16*gam + j)*98 + k
                    # tab16[p, (c*8+gam)*98 + k] = s[16*e + j], j = p & 15
                    tab16 = keep.tile([P, NE], F32)
                    s_flat = s_full[:].rearrange("a b -> (a b)")
                    engines = [nc.sync, nc.scalar]
                    for g2 in range(8):
                        src = bass.AP(
                            s_flat.tensor,
                            0,
                            [[RPP, 16], [VPC, 8], [16 * RPP, 8], [1, RPP]],
                        )
                        engines[g2 % 2].dma_start(
                            tab16[16 * g2 : 16 * g2 + 16, :].rearrange(
                                "p (c g k) -> p c g k", g=8, k=RPP
                            ),
                            src,
                        )

                # ---- stage 4: gather + mask + run-reduce ----
                idx_sb = keep.tile([P, TOK_PER_GROUP // 16], I16)
                nc.sync.dma_start(idx_sb[:], idx_t[:])
                w8_sb = keep.tile([P, 8], F32)
                nc.sync.dma_start(w8_sb[:], w8_t[:])
                rs = keep.tile([P, ROWS_PER_GROUP], F32)

                with tc.tile_pool(name="gat", bufs=gat_bufs) as gat:
                    for t in range(NT):
                        msk_sb = gat.tile([P, NI], mask_dtype, tag="msk", name=f"m{t}")
                        nc.sync.dma_start(
                            msk_sb[:], msk_t[:, t * NI : (t + 1) * NI]
                        )
                        gth = gat.tile([P, NI], F32, tag="gth", name=f"g{t}")
                        nc.gpsimd.ap_gather(
                            gth[:],
                            tab16[:],
                            idx_sb[:, t * (NI // 16) : (t + 1) * (NI // 16)],
                            channels=P,
                            num_elems=NE,
                            d=1,
                            num_idxs=NI,
                        )
                        msel = gat.tile([P, NI], F32, tag="msel", name=f"s{t}")
                        nc.vector.tensor_tensor(
                            out=msel[:], in0=gth[:], in1=msk_sb[:],
                            op=mybir.AluOpType.mult,
                        )
                        nc.vector.tensor_reduce(
                            out=rs[:, t * 32 : (t + 1) * 32].unsqueeze(2),
                            in_=msel[:].rearrange("p (r l) -> p r l", l=200),
                            axis=mybir.AxisListType.X,
                            op=mybir.AluOpType.add,
                        )

                # ---- stage 5: PE group-sum over the 16 partitions of each group ----
                with (
                    tc.tile_pool(name="ps", bufs=1, space="PSUM") as ps,
                    tc.tile_pool(name="fin", bufs=1) as fin,
                ):
                    psum = ps.tile([8, ROWS_PER_GROUP], F32)
                    nc.tensor.matmul(psum[:], w8_sb[:], rs[:])
                    out_sb = fin.tile([8, ROWS_PER_GROUP], F32)
                    nc.any.tensor_copy(out_sb[:], psum[:])
                    nc.sync.dma_start(out_t[:], out_sb[:])
    nc.compile()
    return nc


def make_in_maps(word_idxs, embed_table, weights):
    idx = np.asarray(word_idxs).astype(np.int64)
    tab = np.asarray(embed_table, dtype=np.float32)
    w = np.asarray(weights, dtype=np.float32).reshape(-1)
    tab_pad = np.zeros((VPC * NCORES, D), dtype=np.float32)
    tab_pad[:V] = tab
    w_c = np.ascontiguousarray(
        np.broadcast_to((w / np.float32(L))[None, :], (P, D))
    ).astype(np.float32)
    w8 = np.zeros((P, 8), dtype=np.float32)
    w8[np.arange(P), np.arange(P) >> 4] = 1.0

    # striped vocab-row assignment: v(c, p, k) = c*VPC + 16*(98*(p>>4)+k) + (p&15)
    p_ar = np.arange(P)
    k_ar = np.arange(RPP)
    vmat = 16 * (RPP * (p_ar[:, None] >> 4) + k_ar[None, :]) + (p_ar[:, None] & 15)

    in_maps = []
    for c in range(NCORES):
        tab_c = np.ascontiguousarray(
            tab_pad[c * VPC + vmat].reshape(P, RPP * D)
        )
        rows = idx[c * ROWS_PER_CORE : (c + 1) * ROWS_PER_CORE]  # [2048, 200]
        Lg = rows.reshape(8, TOK_PER_GROUP)  # group g: rows g*256..., in order
        m = (Lg >> 4).astype(np.int16)  # [8, 51200]
        j = (Lg & 15).astype(np.int64)
        # idx wrap: idx_t[16g+w, t*400+s] = m[g, t*6400 + s*16 + w]
        mw = m.reshape(8, NT, NI // 16, 16)  # [g, t, s, w]
        idx_t = np.ascontiguousarray(
            mw.transpose(0, 3, 1, 2).reshape(P, NT * (NI // 16))
        )
        # mask: msk[16g+u, i] = (j[g, i] == u)
        u = np.arange(16)
        msk = (j[:, None, :] == u[None, :, None])  # [8, 16, 51200]
        import ml_dtypes

        msk_bf = np.ascontiguousarray(
            msk.reshape(P, TOK_PER_GROUP).astype(ml_dtypes.bfloat16)
        )
        in_maps.append(
            {"tab": tab_c, "w": w_c, "idx": idx_t, "msk": msk_bf, "w8": w8}
        )
    return in_maps


def unshard_out(results):
    parts = []
    for c in range(NCORES):
        o = np.asarray(results[c]["out"])  # [8, 256]: row c*2048 + g*256 + n
        parts.append(o.reshape(-1))
    return np.concatenate(parts).reshape(-1, 1).astype(np.float32)


_CACHED_NC = None


def _get_nc():
    global _CACHED_NC
    if _CACHED_NC is None:
        _CACHED_NC = build_program()
    return _CACHED_NC


def run(word_idxs, embed_table, weights, trace=False, **spmd_kwargs):
    nc = _get_nc()
    in_maps = make_in_maps(word_idxs, embed_table, weights)
    res = run_bass_kernel_spmd(
        nc, in_maps, core_ids=list(range(NCORES)), trace=trace, **spmd_kwargs
    )
    out = unshard_out(res.results)
    return out, res


def kernel(word_idxs, embed_table, weights):
    out, _ = run(word_idxs, embed_table, weights, trace=False)
    return out


# revision 4
# speedup vs baseline: 1.0079x; 1.0060x over previous
"""Trainium2 Bass kernel: embedding-lookup -> mean-pool -> dot(weights).

out[b] = sum_l s[idx[b,l]],  s = embed_table @ (weights/L)   (V=100000, D=100)

Gather strategy (per core, 2048 batch rows, 409,600 tokens):
  - s striped 16-way across partitions: tab16[p, e] = s[16e + (p&15)],
    e < 6272 (25KB/partition, identical stripes in each 16-partition group).
  - 8x ap_gather (GPSIMD, all 8 Q7 cores in parallel): each 16-partition
    group g gathers its own token list L_g (its 256 batch rows x 200 tokens)
    by m = v>>4; output [128, 6400] holds, at partition p, s[16*m_i + (p&15)].
  - DVE: multiply by a host-shipped bf16 lane mask (j_i == p&15), reduce
    each row's 200-token run -> rs[128, 256] partial sums per partition.
  - PE: W8^T @ rs with W8[p, m] = (p>>4 == m) sums the 16 partitions of each
    group -> psum [8, 256] = all 2048 row outputs.

Vocab-parallel s precompute (12544 rows/core, strided row assignment so the
local s chunk is already stripe-ordered) + AllGather, as in the classic
data-parallel embedding recipe. Host does layout only: row re-ordering of the
table, index splitting (v>>4, v&15), wrap layouts, concat of outputs.
"""

import os
import sys

import numpy as np

for _p in ("/opt/trn_rl_repo",):
    if os.path.isdir(_p) and _p not in sys.path:
        sys.path.insert(0, _p)

from concourse import bacc, bass, mybir, tile  # noqa: E402
from concourse.bass_utils import run_bass_kernel_spmd  # noqa: E402

F32 = mybir.dt.float32
BF16 = mybir.dt.bfloat16
I16 = mybir.dt.int16
P = 128
NCORES = 8

B, L, D, V = 16384, 200, 100, 100000
RPP = 98  # vocab rows per partition (per core): 128*98*8 = 100352 >= V
VPC = P * RPP  # 12544 vocab rows per core
NE = VPC * NCORES // 16  # 6272 stripe entries per partition
ROWS_PER_CORE = B // NCORES  # 2048
ROWS_PER_GROUP = ROWS_PER_CORE // 8  # 256
TOK_PER_GROUP = ROWS_PER_GROUP * L  # 51200
NI = 6400  # idxs per ap_gather per group (32 rows' runs)
NT = TOK_PER_GROUP // NI  # 8 gather instructions


def build_program(mask_dtype=BF16, gat_bufs=2):
    nc = bacc.Bacc(
        "TRN2", target_bir_lowering=False, debug=False, num_devices=NCORES
    )
    tab_t = nc.dram_tensor("tab", [P, RPP * D], F32, kind="ExternalInput")
    w_t = nc.dram_tensor("w", [P, D], F32, kind="ExternalInput")
    idx_t = nc.dram_tensor("idx", [P, TOK_PER_GROUP // 16], I16, kind="ExternalInput")
    msk_t = nc.dram_tensor("msk", [P, TOK_PER_GROUP], mask_dtype, kind="ExternalInput")
    w8_t = nc.dram_tensor("w8", [P, 8], F32, kind="ExternalInput")
    out_t = nc.dram_tensor("out", [8, ROWS_PER_GROUP], F32, kind="ExternalOutput")

    with tile.TileContext(nc) as tc:
        with tc.tile_pool(name="dr", bufs=1, space="DRAM") as dr:
            with tc.tile_pool(name="keep", bufs=1) as keep:
                # small hot-phase inputs first, on the (otherwise idle) sync
                # queue, so the first gather's deps land as early as possible
                idx_sb = keep.tile([P, TOK_PER_GROUP // 16], I16)
                nc.sync.dma_start(idx_sb[:], idx_t[:])
                w8_sb = keep.tile([P, 8], F32)
                nc.sync.dma_start(w8_sb[:], w8_t[:])
                rs = keep.tile([P, ROWS_PER_GROUP], F32)

                with tc.tile_pool(name="pre", bufs=1) as pre:
                    # ---- stage 1: local striped s chunk = (table slice) @ (w/L);
                    # big table loads on the scalar queue, DVE pipelined in 4
                    # chunks ----
                    tab_sb = pre.tile([P, RPP * D], F32)
                    w_sb = pre.tile([P, D], F32)
                    nc.scalar.dma_start(w_sb[:], w_t[:])
                    s_sb = pre.tile([P, RPP], F32)
                    NCH = 4
                    CH = RPP // NCH  # 24; last chunk takes the remainder
                    bounds = [(h * CH, (h + 1) * CH if h < NCH - 1 else RPP)
                              for h in range(NCH)]
                    for h, (r0, r1) in enumerate(bounds):
                        nc.scalar.dma_start(
                            tab_sb[:, r0 * D : r1 * D], tab_t[:, r0 * D : r1 * D]
                        )
                        prod_sb = pre.tile(
                            [P, (r1 - r0) * D], F32, tag="prod", name=f"pr{h}"
                        )
                        nc.vector.tensor_tensor(
                            out=prod_sb[:].rearrange("p (r d) -> p r d", d=D),
                            in0=tab_sb[:, r0 * D : r1 * D].rearrange(
                                "p (r d) -> p r d", d=D
                            ),
                            in1=w_sb[:].unsqueeze(1).to_broadcast([P, r1 - r0, D]),
                            op=mybir.AluOpType.mult,
                        )
                        nc.vector.tensor_reduce(
                            out=s_sb[:, r0:r1].unsqueeze(2),
                            in_=prod_sb[:].rearrange("p (r d) -> p r d", d=D),
                            axis=mybir.AxisListType.X,
                            op=mybir.AluOpType.add,
                        )

                    # ---- stage 2: AllGather striped s ----
                    s_part = dr.tile([P, RPP], F32)
                    nc.scalar.dma_start(s_part[:], s_sb[:])
                    s_full = dr.tile([NCORES * RPP, P], F32, addr_space="Shared")
                    nc.gpsimd.collective_compute(
                        "AllGather",
                        mybir.AluOpType.bypass,
                        replica_groups=[list(range(NCORES))],
                        ins=[s_part.opt()],
                        outs=[s_full.opt()],
                    )

                    # ---- stage 3: load striped table into SBUF ----
                    # flat s_full element index = c*12544 + (16*gam + j)*98 + k
                    # tab16[p, (c*8+gam)*98 + k] = s[16*e + j], j = p & 15
                    tab16 = keep.tile([P, NE], F32)
                    s_flat = s_full[:].rearrange("a b -> (a b)")
                    engines = [nc.sync, nc.scalar]
                    for g2 in range(8):
                        src = bass.AP(
                            s_flat.tensor,
                            0,
                            [[RPP, 16], [VPC, 8], [16 * RPP, 8], [1, RPP]],
                        )
                        engines[g2 % 2].dma_start(
                            tab16[16 * g2 : 16 * g2 + 16, :].rearrange(
                                "p (c g k) -> p c g k", g=8, k=RPP
                            ),
                            src,
                        )

                # ---- stage 4: gather + mask + run-reduce ----
                with (
                    tc.tile_pool(name="mskp", bufs=4) as mskp,
                    tc.tile_pool(name="gat", bufs=gat_bufs) as gat,
                ):
                    for t in range(NT):
                        msk_sb = mskp.tile([P, NI], mask_dtype, tag="msk", name=f"m{t}")
                        nc.sync.dma_start(
                            msk_sb[:], msk_t[:, t * NI : (t + 1) * NI]
                        )
                        gth = gat.tile([P, NI], F32, tag="gth", name=f"g{t}")
                        nc.gpsimd.ap_gather(
                            gth[:],
                            tab16[:],
                            idx_sb[:, t * (NI // 16) : (t + 1) * (NI // 16)],
                            channels=P,
                            num_elems=NE,
                            d=1,
                            num_idxs=NI,
                        )
                        msel = gat.tile([P, NI], F32, tag="msel", name=f"s{t}")
                        nc.vector.tensor_tensor(
                            out=msel[:], in0=gth[:], in1=msk_sb[:],
                            op=mybir.AluOpType.mult,
                        )
                        nc.vector.tensor_reduce(
                            out=rs[:, t * 32 : (t + 1) * 32].unsqueeze(2),
                            in_=msel[:].rearrange("p (r l) -> p r l", l=200),
                            axis=mybir.AxisListType.X,
                            op=mybir.AluOpType.add,
                        )

                # ---- stage 5: PE group-sum over the 16 partitions of each group ----
                with (
                    tc.tile_pool(name="ps", bufs=1, space="PSUM") as ps,
                    tc.tile_pool(name="fin", bufs=1) as fin,
                ):
                    psum = ps.tile([8, ROWS_PER_GROUP], F32)
                    nc.tensor.matmul(psum[:], w8_sb[:], rs[:])
                    out_sb = fin.tile([8, ROWS_PER_GROUP], F32)
                    nc.any.tensor_copy(out_sb[:], psum[:])
                    nc.sync.dma_start(out_t[:], out_sb[:])
    nc.compile()
    return nc


def make_in_maps(word_idxs, embed_table, weights):
    idx = np.asarray(word_idxs).astype(np.int64)
    tab = np.asarray(embed_table, dtype=np.float32)
    w = np.asarray(weights, dtype=np.float32).reshape(-1)
    tab_pad = np.zeros((VPC * NCORES, D), dtype=np.float32)
    tab_pad[:V] = tab
    w_c = np.ascontiguousarray(
        np.broadcast_to((w / np.float32(L))[None, :], (P, D))
    ).astype(np.float32)
    w8 = np.zeros((P, 8), dtype=np.float32)
    w8[np.arange(P), np.arange(P) >> 4] = 1.0

    # striped vocab-row assignment: v(c, p, k) = c*VPC + 16*(98*(p>>4)+k) + (p&15)
    p_ar = np.arange(P)
    k_ar = np.arange(RPP)
    vmat = 16 * (RPP * (p_ar[:, None] >> 4) + k_ar[None, :]) + (p_ar[:, None] & 15)

    in_maps = []
    for c in range(NCORES):
        tab_c = np.ascontiguousarray(
            tab_pad[c * VPC + vmat].reshape(P, RPP * D)
        )
        rows = idx[c * ROWS_PER_CORE : (c + 1) * ROWS_PER_CORE]  # [2048, 200]
        Lg = rows.reshape(8, TOK_PER_GROUP)  # group g: rows g*256..., in order
        m = (Lg >> 4).astype(np.int16)  # [8, 51200]
        j = (Lg & 15).astype(np.int64)
        # idx wrap: idx_t[16g+w, t*400+s] = m[g, t*6400 + s*16 + w]
        mw = m.reshape(8, NT, NI // 16, 16)  # [g, t, s, w]
        idx_t = np.ascontiguousarray(
            mw.transpose(0, 3, 1, 2).reshape(P, NT * (NI // 16))
        )
        # mask: msk[16g+u, i] = (j[g, i] == u)
        u = np.arange(16)
        msk = (j[:, None, :] == u[None, :, None])  # [8, 16, 51200]
        import ml_dtypes

        msk_bf = np.ascontiguousarray(
            msk.reshape(P, TOK_PER_GROUP).astype(ml_dtypes.bfloat16)
        )
        in_maps.append(
            {"tab": tab_c, "w": w_c, "idx": idx_t, "msk": msk_bf, "w8": w8}
        )
    return in_maps


def unshard_out(results):
    parts = []
    for c in range(NCORES):
        o = np.asarray(results[c]["out"])  # [8, 256]: row c*2048 + g*256 + n
        parts.append(o.reshape(-1))
    return np.concatenate(parts).reshape(-1, 1).astype(np.float32)


_CACHED_NC = None


def _get_nc():
    global _CACHED_NC
    if _CACHED_NC is None:
        _CACHED_NC = build_program()
    return _CACHED_NC


def run(word_idxs, embed_table, weights, trace=False, **spmd_kwargs):
    nc = _get_nc()
    in_maps = make_in_maps(word_idxs, embed_table, weights)
    res = run_bass_kernel_spmd(
        nc, in_maps, core_ids=list(range(NCORES)), trace=trace, **spmd_kwargs
    )
    out = unshard_out(res.results)
    return out, res


def kernel(word_idxs, embed_table, weights):
    out, _ = run(word_idxs, embed_table, weights, trace=False)
    return out
